# revision 21
# baseline (speedup 1.0000x reference)
"""Trainium2 Bass kernel for nn_CCNN (banded continuous-kernel conv).

Math: the reference builds a full (B,L,L) pairwise tensor, runs a tiny
scalar->8x8-matrix MLP on every (i,j) pair, masks to the band
j in [i-5, i-1], and contracts:  x_new[b,i,:] = x[b,i,:] @ sum_j kv[b,i,j].
Only the 5 sub-diagonals survive the band mask, so we evaluate the MLP
only on the 5 offsets o=1..5 per row:  dt_o = t_i - t_{i-o}.

Layout on device (per core, R=256 rows of the flattened (B*L) row axis):
  - hidden dims on partitions, rows on the free dim (256 columns)
  - all 5 offsets are batched into one matmul chain via block-diagonal
    weights: h1 is (5*16=80, R), h2 is (5*32=160 -> split 96+64, R), etc.
  - the o-sum happens inside the final W4 matmul (W4 tiled vertically),
    band masking is folded in as (mask * h3) @ W4 + B4 (x) nmask.
  - the per-row x contraction uses selection-matrix matmuls:
      xe[(c,d), r] = x[c, r]     (partition broadcast via matmul)
      prod = Msum * xe           (elementwise)
      x_new[d, r] = sum_c prod[(c,d), r]   (selection matmul)
"""

import numpy as np

F = 2
KW = 5  # band width (kernel size)
CIN = 8
COUT = 8
H1, H2, H3 = 16, 32, 16
NT = 100  # n_types
B, L = 4, 512
NCORES = 8
R = (B * L) // NCORES  # 256 rows per core

# number of o-offsets in the A/B partition-split of the h2/h3 stages
# (5*32 = 160 > 128 partitions, so the o axis splits 3 + 2)
OA, OB = 3, 2

TRACE = False
LAST_RESULTS = None

_cache = {}


def _layer_items(f):
    """(name, partitions, cols) for one layer's packed weights, in order."""
    return [
        (f"W1A{f}", KW, OA * H1),            # block-diag W1, offsets 0..2: (5, 48)
        (f"B1A{f}", OA * H1, 1),             # (48, 1)
        (f"W1B{f}", KW, OB * H1),            # block-diag W1, offsets 3..4: (5, 32)
        (f"B1B{f}", OB * H1, 1),             # (32, 1)
        (f"W2A{f}", OA * H1, OA * H2),       # (48, 96)
        (f"B2A{f}", OA * H2, 1),             # (96, 1)
        (f"W2B{f}", OB * H1, OB * H2),       # (32, 64)
        (f"B2B{f}", OB * H2, 1),             # (64, 1)
        (f"W3A{f}", OA * H2, OA * H3),       # (96, 48)
        (f"B3A{f}", OA * H3, 1),             # (48, 1)
        (f"W3B{f}", OB * H2, OB * H3),       # (64, 32)
        (f"B3B{f}", OB * H3, 1),             # (32, 1)
        (f"W4A{f}", OA * H3, CIN * COUT),    # (48, 64)
        (f"W4B{f}", OB * H3, CIN * COUT),    # (32, 64)
        (f"B4row{f}", 1, CIN * COUT),        # (1, 64)
    ]


def _pack_layouts():
    """Two packed-constant DRAM tensors: pack0 (layer0 + shared), pack1."""
    pack0 = [
        ("embX", NT, CIN * COUT),   # emb tiled over d: (100, 64)
        ("SelX", CIN * COUT, CIN * COUT),
        ("sel8", CIN * COUT, COUT),
    ] + _layer_items(0)
    pack1 = _layer_items(1)

    def alloc(items):
        cols = {}
        col = 0
        for name, p, w in items:
            cols[name] = (p, col, w)
            col += w
        return cols, col

    c0, w0 = alloc(pack0)
    c1, w1 = alloc(pack1)
    return (c0, w0), (c1, w1)


def _build_pack_arrays(emb, W1, B1, W2, B2, W3, B3, W4, B4):
    (c0, w0), (c1, w1) = _pack_layouts()
    pack0 = np.zeros((128, w0), np.float32)
    pack1 = np.zeros((128, w1), np.float32)

    def put(pack, cols, name, arr):
        p, col, w = cols[name]
        assert arr.shape == (p, w), (name, arr.shape, (p, w))
        pack[:p, col : col + w] = arr

    # embX[t, c*8+d] = emb[t, c]
    put(pack0, c0, "embX", np.repeat(emb.astype(np.float32), COUT, axis=1))
    # SelX[(c',d'), (c,d)] = 1 if d' == c  -> xe_next[(c,d)] = sum_c' prod[(c',c)]
    selx = np.zeros((CIN * COUT, CIN * COUT), np.float32)
    for cp in range(CIN):
        for dp in range(COUT):
            for d in range(COUT):
                selx[cp * COUT + dp, dp * COUT + d] = 1.0
    put(pack0, c0, "SelX", selx)
    # sel8[(c,d), d'] = (d == d')
    put(pack0, c0, "sel8", np.tile(np.eye(COUT, dtype=np.float32), (CIN, 1)))

    for f, (pack, cols) in enumerate([(pack0, c0), (pack1, c1)]):
        w1f = W1[f].reshape(H1).astype(np.float32)
        w2f = W2[f].astype(np.float32)
        w3f = W3[f].astype(np.float32)
        w4f = W4[f].astype(np.float32)

        w1a = np.zeros((KW, OA * H1), np.float32)
        for o in range(OA):
            w1a[o, o * H1 : (o + 1) * H1] = w1f
        put(pack, cols, f"W1A{f}", w1a)
        put(pack, cols, f"B1A{f}", np.tile(B1[f], OA)[:, None].astype(np.float32))
        w1b = np.zeros((KW, OB * H1), np.float32)
        for o in range(OB):
            w1b[OA + o, o * H1 : (o + 1) * H1] = w1f
        put(pack, cols, f"W1B{f}", w1b)
        put(pack, cols, f"B1B{f}", np.tile(B1[f], OB)[:, None].astype(np.float32))

        w2a = np.zeros((OA * H1, OA * H2), np.float32)
        for o in range(OA):
            w2a[o * H1 : (o + 1) * H1, o * H2 : (o + 1) * H2] = w2f
        put(pack, cols, f"W2A{f}", w2a)
        put(pack, cols, f"B2A{f}", np.tile(B2[f], OA)[:, None].astype(np.float32))
        w2b = np.zeros((OB * H1, OB * H2), np.float32)
        for o in range(OB):
            w2b[o * H1 : (o + 1) * H1, o * H2 : (o + 1) * H2] = w2f
        put(pack, cols, f"W2B{f}", w2b)
        put(pack, cols, f"B2B{f}", np.tile(B2[f], OB)[:, None].astype(np.float32))

        w3a = np.zeros((OA * H2, OA * H3), np.float32)
        for o in range(OA):
            w3a[o * H2 : (o + 1) * H2, o * H3 : (o + 1) * H3] = w3f
        put(pack, cols, f"W3A{f}", w3a)
        put(pack, cols, f"B3A{f}", np.tile(B3[f], OA)[:, None].astype(np.float32))
        w3b = np.zeros((OB * H2, OB * H3), np.float32)
        for o in range(OB):
            w3b[o * H2 : (o + 1) * H2, o * H3 : (o + 1) * H3] = w3f
        put(pack, cols, f"W3B{f}", w3b)
        put(pack, cols, f"B3B{f}", np.tile(B3[f], OB)[:, None].astype(np.float32))

        put(pack, cols, f"W4A{f}", np.tile(w4f, (OA, 1)))
        put(pack, cols, f"W4B{f}", np.tile(w4f, (OB, 1)))
        put(pack, cols, f"B4row{f}", B4[f][None, :].astype(np.float32))

    return pack0, pack1


def _build_nc():
    import concourse.bacc as bacc
    import concourse.mybir as mybir
    from concourse.tile import TileContext

    F32 = mybir.dt.float32
    RELU = mybir.ActivationFunctionType.Relu
    ADD = mybir.AluOpType.add
    MAX = mybir.AluOpType.max

    (c0, w0), (c1, w1) = _pack_layouts()

    nc = bacc.Bacc("TRN2", debug=False)
    pack0_d = nc.dram_tensor("pack0", (128, w0), F32, kind="ExternalInput")
    pack1_d = nc.dram_tensor("pack1", (128, w1), F32, kind="ExternalInput")
    onehot_d = nc.dram_tensor("onehot", (NT, R), F32, kind="ExternalInput")
    # mask80 rows: 0..47 = offsets 0..2 (x16), 48..79 = offsets 3..4 (x16)
    mask_d = nc.dram_tensor("mask80", (KW * H3, R), F32, kind="ExternalInput")
    tvec_d = nc.dram_tensor("tvec", (2 * KW + 1, R), F32, kind="ExternalInput")
    out_d = nc.dram_tensor("out", (CIN, R), F32, kind="ExternalOutput")

    with TileContext(nc) as tc:
        with (
            tc.tile_pool(name="const", bufs=1) as cpool,
            tc.tile_pool(name="work", bufs=2) as wpool,
            tc.tile_pool(name="psum", bufs=2, space="PSUM") as ppool,
        ):
            pack0 = cpool.tile([128, w0], F32, tag="pack0")
            nc.gpsimd.dma_start(out=pack0, in_=pack0_d.ap())
            pack1 = cpool.tile([128, w1], F32, tag="pack1")
            nc.gpsimd.dma_start(out=pack1, in_=pack1_d.ap())
            onehot = cpool.tile([NT, R], F32, tag="onehot")
            nc.gpsimd.dma_start(out=onehot, in_=onehot_d.ap())
            maskA = cpool.tile([OA * H3, R], F32, tag="maskA")
            nc.gpsimd.dma_start(out=maskA, in_=mask_d.ap()[0 : OA * H3, :])
            maskB = cpool.tile([OB * H3, R], F32, tag="maskB")
            nc.gpsimd.dma_start(out=maskB, in_=mask_d.ap()[OA * H3 : KW * H3, :])
            # compute reads must start at a x32 partition: split tvec rows
            # into three base-0 tiles
            nmask_t = cpool.tile([1, R], F32, tag="nmask")
            nc.gpsimd.dma_start(out=nmask_t, in_=tvec_d.ap()[0:1, :])
            tcur_t = cpool.tile([KW, R], F32, tag="tcur")
            nc.gpsimd.dma_start(out=tcur_t, in_=tvec_d.ap()[1 : 1 + KW, :])
            tprev_t = cpool.tile([KW, R], F32, tag="tprev")
            nc.gpsimd.dma_start(out=tprev_t, in_=tvec_d.ap()[1 + KW : 1 + 2 * KW, :])

            packs = {0: (pack0, c0), 1: (pack1, c1)}

            def cslice(f, name):
                pk, cols = packs[f]
                p, col, w = cols[name]
                return pk[0:p, col : col + w]

            # dt[o, r] = t_i - t_{i-1-o} (garbage where masked; masked later)
            dt = wpool.tile([KW, R], F32, tag="dt")
            nc.vector.tensor_sub(out=dt, in0=tcur_t, in1=tprev_t)
            nmask = nmask_t

            # xe[(c,d), r] = x0[c, r] = (emb gather), partition-broadcast over d
            xe_ps = ppool.tile([CIN * COUT, R], F32, tag="xe_ps")
            nc.tensor.matmul(xe_ps, cslice(0, "embX"), onehot, start=True, stop=True)
            xe = wpool.tile([CIN * COUT, R], F32, tag="xe")
            nc.scalar.copy(out=xe, in_=xe_ps)

            for f in range(F):
                # ---- the 5-offset MLP, block-diagonal over o, split A(3)+B(2) ----
                h1psA = ppool.tile([OA * H1, R], F32, tag="mm", bufs=4)
                nc.tensor.matmul(h1psA, cslice(f, f"W1A{f}"), dt, start=True, stop=True)
                h1psB = ppool.tile([OB * H1, R], F32, tag="mm", bufs=4)
                nc.tensor.matmul(h1psB, cslice(f, f"W1B{f}"), dt, start=True, stop=True)
                h1A = wpool.tile([OA * H1, R], F32, tag="h1A")
                nc.scalar.activation(out=h1A, in_=h1psA, func=RELU, bias=cslice(f, f"B1A{f}"))
                h1B = wpool.tile([OB * H1, R], F32, tag="h1B")
                nc.vector.tensor_scalar(h1B, h1psB, cslice(f, f"B1B{f}"), 0.0, ADD, MAX)

                h2psA = ppool.tile([OA * H2, R], F32, tag="mm", bufs=4)
                nc.tensor.matmul(h2psA, cslice(f, f"W2A{f}"), h1A, start=True, stop=True)
                h2psB = ppool.tile([OB * H2, R], F32, tag="mm", bufs=4)
                nc.tensor.matmul(h2psB, cslice(f, f"W2B{f}"), h1B, start=True, stop=True)
                h2A = wpool.tile([OA * H2, R], F32, tag="h2A")
                nc.scalar.activation(out=h2A, in_=h2psA, func=RELU, bias=cslice(f, f"B2A{f}"))
                h2B = wpool.tile([OB * H2, R], F32, tag="h2B")
                nc.vector.tensor_scalar(h2B, h2psB, cslice(f, f"B2B{f}"), 0.0, ADD, MAX)

                h3psA = ppool.tile([OA * H3, R], F32, tag="mm", bufs=4)
                nc.tensor.matmul(h3psA, cslice(f, f"W3A{f}"), h2A, start=True, stop=True)
                h3psB = ppool.tile([OB * H3, R], F32, tag="mm", bufs=4)
                nc.tensor.matmul(h3psB, cslice(f, f"W3B{f}"), h2B, start=True, stop=True)
                h3A = wpool.tile([OA * H3, R], F32, tag="h3A")
                nc.scalar.activation(out=h3A, in_=h3psA, func=RELU, bias=cslice(f, f"B3A{f}"))
                h3B = wpool.tile([OB * H3, R], F32, tag="h3B")
                nc.vector.tensor_scalar(h3B, h3psB, cslice(f, f"B3B{f}"), 0.0, ADD, MAX)

                # band mask folded in before the o-summing W4 matmul
                h3mA = wpool.tile([OA * H3, R], F32, tag="h3mA")
                nc.vector.tensor_mul(out=h3mA, in0=h3A, in1=maskA)
                h3mB = wpool.tile([OB * H3, R], F32, tag="h3mB")
                nc.vector.tensor_mul(out=h3mB, in0=h3B, in1=maskB)

                # Msum[(c,d), r] = sum_o kv_o + B4 * nmask  (o-sum inside matmul)
                msum = ppool.tile([CIN * COUT, R], F32, tag="msum")
                nc.tensor.matmul(msum, cslice(f, f"W4A{f}"), h3mA, start=True, stop=False)
                nc.tensor.matmul(msum, cslice(f, f"W4B{f}"), h3mB, start=False, stop=False)
                nc.tensor.matmul(msum, cslice(f, f"B4row{f}"), nmask, start=False, stop=True)

                # x contraction: prod = Msum * xe; then selection matmul
                prod = wpool.tile([CIN * COUT, R], F32, tag="prod")
                nc.vector.tensor_mul(out=prod, in0=msum, in1=xe)

                if f < F - 1:
                    xe_ps2 = ppool.tile([CIN * COUT, R], F32, tag="xe_ps")
                    nc.tensor.matmul(xe_ps2, cslice(0, "SelX"), prod, start=True, stop=True)
                    xe = wpool.tile([CIN * COUT, R], F32, tag="xe")
                    nc.scalar.copy(out=xe, in_=xe_ps2)
                else:
                    out_ps = ppool.tile([CIN, R], F32, tag="xe_ps")
                    nc.tensor.matmul(out_ps, cslice(0, "sel8"), prod, start=True, stop=True)
                    xout = wpool.tile([CIN, R], F32, tag="xout")
                    nc.vector.tensor_copy(out=xout, in_=out_ps)
                    nc.sync.dma_start(out=out_d.ap(), in_=xout)

    nc.finalize()
    return nc


def _per_core_inputs(times, features, core):
    rows = np.arange(core * R, (core + 1) * R)
    b = rows // L
    i = rows % L

    tcur = times[b, i].astype(np.float32)
    tc5 = np.tile(tcur, (KW, 1))
    tp5 = np.zeros((KW, R), np.float32)
    mask = np.zeros((KW, R), np.float32)
    for o in range(1, KW + 1):
        valid = i >= o
        tp5[o - 1, valid] = times[b[valid], i[valid] - o]
        mask[o - 1, valid] = 1.0
    mask80 = np.repeat(mask, H3, axis=0)  # (80, R): partition (o*16 + h)
    nmask = mask.sum(axis=0, keepdims=True)
    tvec = np.ascontiguousarray(np.concatenate([nmask, tc5, tp5], axis=0))

    feat = features[b, i].astype(np.int64)
    onehot = (feat[None, :] == np.arange(NT)[:, None]).astype(np.float32)
    return tvec, mask80, onehot


def kernel(times, features, emb, W1, B1, W2, B2, W3, B3, W4, B4):
    global LAST_RESULTS
    from concourse.bass_utils import run_bass_kernel_spmd

    times = np.asarray(times, dtype=np.float32)
    features = np.asarray(features)
    emb = np.asarray(emb, dtype=np.float32)
    W1, B1 = np.asarray(W1, np.float32), np.asarray(B1, np.float32)
    W2, B2 = np.asarray(W2, np.float32), np.asarray(B2, np.float32)
    W3, B3 = np.asarray(W3, np.float32), np.asarray(B3, np.float32)
    W4, B4 = np.asarray(W4, np.float32), np.asarray(B4, np.float32)

    if "nc" not in _cache:
        _cache["nc"] = _build_nc()
    nc = _cache["nc"]

    pack0, pack1 = _build_pack_arrays(emb, W1, B1, W2, B2, W3, B3, W4, B4)

    in_maps = []
    for core in range(NCORES):
        tvec, mask80, onehot = _per_core_inputs(times, features, core)
        in_maps.append(
            {
                "pack0": pack0,
                "pack1": pack1,
                "onehot": onehot,
                "mask80": mask80,
                "tvec": tvec,
            }
        )

    res = run_bass_kernel_spmd(nc, in_maps, list(range(NCORES)), trace=TRACE)
    LAST_RESULTS = res

    out = np.zeros((B * L, CIN), np.float32)
    for core in range(NCORES):
        out[core * R : (core + 1) * R, :] = res.results[core]["out"].T
    return out.reshape(B, L, CIN)


# revision 25
# speedup vs baseline: 1.2723x; 1.2723x over previous
"""Trainium2 Bass kernel for nn_CCNN (banded continuous-kernel conv).

Math: the reference builds a full (B,L,L) pairwise tensor, runs a tiny
scalar->8x8-matrix MLP on every (i,j) pair, masks to the band
j in [i-5, i-1], and contracts:  x_new[b,i,:] = x[b,i,:] @ sum_j kv[b,i,j].
Only the 5 sub-diagonals survive the band mask, so we evaluate the MLP
only on the 5 offsets o=1..5 per row:  dt_o = t_i - t_{i-o}.

Layout on device (per core, R=256 rows of the flattened (B*L) row axis):
  - hidden dims on partitions, rows on the free dim (256 columns)
  - all 5 offsets are batched into one matmul chain via block-diagonal
    weights. The o axis splits 3+2 (h2 = 5*32 = 160 > 128 partitions);
    the B-half (offsets 3..4) lives at base partition 64 so that the
    PE quadrant rule (lhsT/rhs base in {0,32,64}, equal) is satisfied
    inside shared 96-partition tiles.
  - stage padding rows 48..63 are reused to fold the +B4*nmask bias
    term into the W4 matmul: h3[48,:]=1 (DMA'd ones), mask[48,:]=nmask,
    W4pad[48,:]=B4.
  - the per-row x contraction uses selection-matrix matmuls:
      xe[(c,d), r] = x[c, r]     (partition broadcast via matmul)
      prod = Msum * xe           (elementwise)
      x_new[d, r] = sum_c prod[(c,d), r]   (selection matmul)
"""

import numpy as np

F = 2
KW = 5  # band width (kernel size)
CIN = 8
COUT = 8
H1, H2, H3 = 16, 32, 16
NT = 100  # n_types
B, L = 4, 512
NCORES = 8
R = (B * L) // NCORES  # 256 rows per core

# offsets 0..2 are the A-half (base partition 0), 3..4 the B-half (base 64)
OA, OB = 3, 2

TRACE = False
LAST_RESULTS = None

_cache = {}


def _layer_items(f):
    """(name, partitions, cols) for one layer's packed weights, in order.

    W2B sits at base partition 64 (its rhs h1[64:96] is at base 64 and the
    PE requires equal lhsT/rhs base partitions); every other block at 0.
    """
    return [
        (f"W1pad{f}", KW, 96, 0),        # cols 0:48 = W1A blockdiag, 64:96 = W1B
        (f"B1pad{f}", 96, 1, 0),         # rows 48:64 zero
        (f"W2A{f}", OA * H1, OA * H2, 0),    # (48, 96)
        (f"B2A{f}", OA * H2, 1, 0),          # (96, 1)
        (f"W2B{f}", OB * H1, OB * H2, 64),   # (32, 64) @ base 64
        (f"B2B{f}", OB * H2, 1, 0),          # (64, 1)
        (f"W3A{f}", OA * H2, OA * H3, 0),    # (96, 48)
        (f"B3A{f}", OA * H3, 1, 0),          # (48, 1)
        (f"W3B{f}", OB * H2, OB * H3, 0),    # (64, 32)
        (f"B3B{f}", OB * H3, 1, 0),          # (32, 1)
        (f"W4pad{f}", 96, CIN * COUT, 0),    # rows 0:48 W4A, 48 B4, 49:64 0, 64:96 W4B
    ]


def _pack_layouts():
    """Packed-constant DRAM tensors: pack0 (layer0 + shared), pack1."""
    pack0 = [
        ("embX", NT, CIN * COUT, 0),   # emb tiled over d: (100, 64)
        ("SelX", CIN * COUT, CIN * COUT, 0),
        ("sel8", CIN * COUT, COUT, 0),
    ] + _layer_items(0)
    pack1 = _layer_items(1)

    def alloc(items):
        cols = {}
        col = 0
        for name, p, w, base in items:
            cols[name] = (p, col, w, base)
            col += w
        return cols, col

    c0, w0 = alloc(pack0)
    c1, w1 = alloc(pack1)
    return (c0, w0), (c1, w1)


def _build_pack_arrays(emb, W1, B1, W2, B2, W3, B3, W4, B4):
    (c0, w0), (c1, w1) = _pack_layouts()
    pack0 = np.zeros((128, w0), np.float32)
    pack1 = np.zeros((128, w1), np.float32)

    def put(pack, cols, name, arr):
        p, col, w, base = cols[name]
        assert arr.shape == (p, w), (name, arr.shape, (p, w))
        pack[base : base + p, col : col + w] = arr

    # embX[t, c*8+d] = emb[t, c]
    put(pack0, c0, "embX", np.repeat(emb.astype(np.float32), COUT, axis=1))
    # SelX[(c',d'), (c,d)] = 1 if d' == c  -> xe_next[(c,d)] = sum_c' prod[(c',c)]
    selx = np.zeros((CIN * COUT, CIN * COUT), np.float32)
    for cp in range(CIN):
        for dp in range(COUT):
            for d in range(COUT):
                selx[cp * COUT + dp, dp * COUT + d] = 1.0
    put(pack0, c0, "SelX", selx)
    # sel8[(c,d), d'] = (d == d')
    put(pack0, c0, "sel8", np.tile(np.eye(COUT, dtype=np.float32), (CIN, 1)))

    for f, (pack, cols) in enumerate([(pack0, c0), (pack1, c1)]):
        w1f = W1[f].reshape(H1).astype(np.float32)
        w2f = W2[f].astype(np.float32)
        w3f = W3[f].astype(np.float32)
        w4f = W4[f].astype(np.float32)

        # W1pad: (5, 96): offset o row -> h1 block; A cols 0:48, B cols 64:96
        w1p = np.zeros((KW, 96), np.float32)
        for o in range(OA):
            w1p[o, o * H1 : (o + 1) * H1] = w1f
        for o in range(OB):
            w1p[OA + o, 64 + o * H1 : 64 + (o + 1) * H1] = w1f
        put(pack, cols, f"W1pad{f}", w1p)
        b1p = np.zeros((96, 1), np.float32)
        b1p[0:48, 0] = np.tile(B1[f], OA)
        b1p[64:96, 0] = np.tile(B1[f], OB)
        put(pack, cols, f"B1pad{f}", b1p)

        w2a = np.zeros((OA * H1, OA * H2), np.float32)
        for o in range(OA):
            w2a[o * H1 : (o + 1) * H1, o * H2 : (o + 1) * H2] = w2f
        put(pack, cols, f"W2A{f}", w2a)
        put(pack, cols, f"B2A{f}", np.tile(B2[f], OA)[:, None].astype(np.float32))
        w2b = np.zeros((OB * H1, OB * H2), np.float32)
        for o in range(OB):
            w2b[o * H1 : (o + 1) * H1, o * H2 : (o + 1) * H2] = w2f
        put(pack, cols, f"W2B{f}", w2b)
        put(pack, cols, f"B2B{f}", np.tile(B2[f], OB)[:, None].astype(np.float32))

        w3a = np.zeros((OA * H2, OA * H3), np.float32)
        for o in range(OA):
            w3a[o * H2 : (o + 1) * H2, o * H3 : (o + 1) * H3] = w3f
        put(pack, cols, f"W3A{f}", w3a)
        put(pack, cols, f"B3A{f}", np.tile(B3[f], OA)[:, None].astype(np.float32))
        w3b = np.zeros((OB * H2, OB * H3), np.float32)
        for o in range(OB):
            w3b[o * H2 : (o + 1) * H2, o * H3 : (o + 1) * H3] = w3f
        put(pack, cols, f"W3B{f}", w3b)
        put(pack, cols, f"B3B{f}", np.tile(B3[f], OB)[:, None].astype(np.float32))

        # W4pad: rows 0:48 = W4 tiled x3, row 48 = B4 (pairs with the ones
        # row DMA'd into h3[48] and nmask in mask[48]), 49:64 = 0,
        # rows 64:96 = W4 tiled x2
        w4p = np.zeros((96, CIN * COUT), np.float32)
        w4p[0:48] = np.tile(w4f, (OA, 1))
        w4p[48] = B4[f]
        w4p[64:96] = np.tile(w4f, (OB, 1))
        put(pack, cols, f"W4pad{f}", w4p)

    return pack0, pack1


def _build_nc():
    import concourse.bacc as bacc
    import concourse.mybir as mybir
    from concourse.tile import TileContext

    F32 = mybir.dt.float32
    RELU = mybir.ActivationFunctionType.Relu
    ADD = mybir.AluOpType.add
    MAX = mybir.AluOpType.max

    (c0, w0), (c1, w1) = _pack_layouts()

    nc = bacc.Bacc("TRN2", debug=False)
    pack0_d = nc.dram_tensor("pack0", (128, w0), F32, kind="ExternalInput")
    pack1_d = nc.dram_tensor("pack1", (128, w1), F32, kind="ExternalInput")
    onehot_d = nc.dram_tensor("onehot", (NT, R), F32, kind="ExternalInput")
    # mask96 rows: 0:48 = offsets 0..2 (x16), 48 = nmask, 49:64 = 0,
    # 64:96 = offsets 3..4 (x16)
    mask_d = nc.dram_tensor("mask96", (96, R), F32, kind="ExternalInput")
    # tvec rows: 0..4 = t_i (x5), 5..9 = t_{i-o}, 10..25 = ones
    tvec_d = nc.dram_tensor("tvec", (2 * KW + 16, R), F32, kind="ExternalInput")
    out_d = nc.dram_tensor("out", (CIN, R), F32, kind="ExternalOutput")

    with TileContext(nc) as tc:
        with (
            tc.tile_pool(name="const", bufs=1) as cpool,
            tc.tile_pool(name="work", bufs=2) as wpool,
            tc.tile_pool(name="psum", bufs=2, space="PSUM") as ppool,
        ):
            pack0 = cpool.tile([128, w0], F32, tag="pack0")
            nc.sync.dma_start(out=pack0, in_=pack0_d.ap())
            pack1 = cpool.tile([128, w1], F32, tag="pack1")
            nc.sync.dma_start(out=pack1, in_=pack1_d.ap())
            onehot = cpool.tile([NT, R], F32, tag="onehot")
            nc.sync.dma_start(out=onehot, in_=onehot_d.ap())
            mask96 = cpool.tile([96, R], F32, tag="mask96")
            nc.sync.dma_start(out=mask96, in_=mask_d.ap())
            tcur_t = cpool.tile([KW, R], F32, tag="tcur")
            nc.sync.dma_start(out=tcur_t, in_=tvec_d.ap()[0:KW, :])
            tprev_t = cpool.tile([KW, R], F32, tag="tprev")
            nc.sync.dma_start(out=tprev_t, in_=tvec_d.ap()[KW : 2 * KW, :])

            packs = {0: (pack0, c0), 1: (pack1, c1)}

            def cslice(f, name):
                pk, cols = packs[f]
                p, col, w, base = cols[name]
                return pk[base : base + p, col : col + w]

            # dt[o, r] = t_i - t_{i-1-o} (garbage where masked; masked later)
            dt = wpool.tile([KW, R], F32, tag="dt")
            nc.vector.tensor_sub(out=dt, in0=tcur_t, in1=tprev_t)

            # xe[(c,d), r] = x0[c, r] = (emb gather), partition-broadcast over d
            xe_ps = ppool.tile([CIN * COUT, R], F32, tag="xe_ps")
            nc.tensor.matmul(xe_ps, cslice(0, "embX"), onehot, start=True, stop=True)
            xe = wpool.tile([CIN * COUT, R], F32, tag="xe")
            nc.scalar.copy(out=xe, in_=xe_ps)

            for f in range(F):
                # ---- the 5-offset MLP, block-diagonal over o (A@0, B@64) ----
                h1ps = ppool.tile([96, R], F32, tag="mm", bufs=4)
                nc.tensor.matmul(h1ps, cslice(f, f"W1pad{f}"), dt, start=True, stop=True)
                h1 = wpool.tile([96, R], F32, tag="h1")
                nc.scalar.activation(out=h1, in_=h1ps, func=RELU, bias=cslice(f, f"B1pad{f}"))

                h2psA = ppool.tile([OA * H2, R], F32, tag="mm", bufs=4)
                nc.tensor.matmul(h2psA, cslice(f, f"W2A{f}"), h1[0 : OA * H1, :], start=True, stop=True)
                h2psB = ppool.tile([OB * H2, R], F32, tag="mm", bufs=4)
                nc.tensor.matmul(h2psB, cslice(f, f"W2B{f}"), h1[64 : 64 + OB * H1, :], start=True, stop=True)
                h2A = wpool.tile([OA * H2, R], F32, tag="h2A")
                nc.scalar.activation(out=h2A, in_=h2psA, func=RELU, bias=cslice(f, f"B2A{f}"))
                h2B = wpool.tile([OB * H2, R], F32, tag="h2B")
                nc.vector.tensor_scalar(h2B, h2psB, cslice(f, f"B2B{f}"), 0.0, ADD, MAX)

                # h3: A-half at base 0 of a shared 96-partition psum/sbuf pair,
                # B-half at base 64; rows 48:64 of h3 get DMA'd ones (row 48
                # pairs with nmask/B4 to fold the bias term into the W4 matmul)
                h3ps = ppool.tile([96, R], F32, tag="mm", bufs=4)
                nc.tensor.matmul(h3ps[0 : OA * H3, :], cslice(f, f"W3A{f}"), h2A, start=True, stop=True)
                nc.tensor.matmul(h3ps[64 : 64 + OB * H3, :], cslice(f, f"W3B{f}"), h2B, start=True, stop=True)
                h3 = wpool.tile([96, R], F32, tag="h3")
                nc.sync.dma_start(out=h3[48:64, :], in_=tvec_d.ap()[2 * KW : 2 * KW + 16, :])
                nc.scalar.activation(out=h3[0 : OA * H3, :], in_=h3ps[0 : OA * H3, :], func=RELU, bias=cslice(f, f"B3A{f}"))
                nc.vector.tensor_scalar(h3[64 : 64 + OB * H3, :], h3ps[64 : 64 + OB * H3, :], cslice(f, f"B3B{f}"), 0.0, ADD, MAX)

                # band mask (+ nmask row) folded in before the W4 matmul
                h3m = wpool.tile([96, R], F32, tag="h3m")
                nc.vector.tensor_mul(out=h3m, in0=h3, in1=mask96)

                # Msum[(c,d), r] = sum_o kv_o + B4 * nmask, all in one matmul
                msum = ppool.tile([CIN * COUT, R], F32, tag="msum")
                nc.tensor.matmul(msum, cslice(f, f"W4pad{f}"), h3m, start=True, stop=True)

                # x contraction: prod = Msum * xe; then selection matmul
                prod = wpool.tile([CIN * COUT, R], F32, tag="prod")
                nc.vector.tensor_mul(out=prod, in0=msum, in1=xe)

                if f < F - 1:
                    xe_ps2 = ppool.tile([CIN * COUT, R], F32, tag="xe_ps")
                    nc.tensor.matmul(xe_ps2, cslice(0, "SelX"), prod, start=True, stop=True)
                    xe = wpool.tile([CIN * COUT, R], F32, tag="xe")
                    nc.scalar.copy(out=xe, in_=xe_ps2)
                else:
                    out_ps = ppool.tile([CIN, R], F32, tag="xe_ps")
                    nc.tensor.matmul(out_ps, cslice(0, "sel8"), prod, start=True, stop=True)
                    xout = wpool.tile([CIN, R], F32, tag="xout")
                    nc.vector.tensor_copy(out=xout, in_=out_ps)
                    nc.sync.dma_start(out=out_d.ap(), in_=xout)

    nc.finalize()
    return nc


def _per_core_inputs(times, features, core):
    rows = np.arange(core * R, (core + 1) * R)
    b = rows // L
    i = rows % L

    tcur = times[b, i].astype(np.float32)
    tc5 = np.tile(tcur, (KW, 1))
    tp5 = np.zeros((KW, R), np.float32)
    mask = np.zeros((KW, R), np.float32)
    for o in range(1, KW + 1):
        valid = i >= o
        tp5[o - 1, valid] = times[b[valid], i[valid] - o]
        mask[o - 1, valid] = 1.0
    mask96 = np.zeros((96, R), np.float32)
    mask96[0 : OA * H3] = np.repeat(mask[:OA], H3, axis=0)  # partition (o*16+h)
    mask96[48] = mask.sum(axis=0)  # nmask row (pairs with ones/B4 at 48)
    mask96[64 : 64 + OB * H3] = np.repeat(mask[OA:], H3, axis=0)
    ones = np.ones((16, R), np.float32)
    tvec = np.ascontiguousarray(np.concatenate([tc5, tp5, ones], axis=0))

    feat = features[b, i].astype(np.int64)
    onehot = (feat[None, :] == np.arange(NT)[:, None]).astype(np.float32)
    return tvec, mask96, onehot


def kernel(times, features, emb, W1, B1, W2, B2, W3, B3, W4, B4):
    global LAST_RESULTS
    from concourse.bass_utils import run_bass_kernel_spmd

    times = np.asarray(times, dtype=np.float32)
    features = np.asarray(features)
    emb = np.asarray(emb, dtype=np.float32)
    W1, B1 = np.asarray(W1, np.float32), np.asarray(B1, np.float32)
    W2, B2 = np.asarray(W2, np.float32), np.asarray(B2, np.float32)
    W3, B3 = np.asarray(W3, np.float32), np.asarray(B3, np.float32)
    W4, B4 = np.asarray(W4, np.float32), np.asarray(B4, np.float32)

    if "nc" not in _cache:
        _cache["nc"] = _build_nc()
    nc = _cache["nc"]

    pack0, pack1 = _build_pack_arrays(emb, W1, B1, W2, B2, W3, B3, W4, B4)

    in_maps = []
    for core in range(NCORES):
        tvec, mask96, onehot = _per_core_inputs(times, features, core)
        in_maps.append(
            {
                "pack0": pack0,
                "pack1": pack1,
                "onehot": onehot,
                "mask96": mask96,
                "tvec": tvec,
            }
        )

    res = run_bass_kernel_spmd(nc, in_maps, list(range(NCORES)), trace=TRACE)
    LAST_RESULTS = res

    out = np.zeros((B * L, CIN), np.float32)
    for core in range(NCORES):
        out[core * R : (core + 1) * R, :] = res.results[core]["out"].T
    return out.reshape(B, L, CIN)


# revision 28
# speedup vs baseline: 1.8044x; 1.4183x over previous
"""Trainium2 Bass kernel for nn_CCNN (banded continuous-kernel conv).

Math: the reference builds a full (B,L,L) pairwise tensor, runs a tiny
scalar->8x8-matrix MLP on every (i,j) pair, masks to the band
j in [i-5, i-1], and contracts:  x_new[b,i,:] = x[b,i,:] @ sum_j kv[b,i,j].
Only the 5 sub-diagonals survive the band mask, so we evaluate the MLP
only on the 5 offsets o=1..5 per row:  dt_o = t_i - t_{i-o}.

Layout on device (per core, R=256 rows of the flattened (B*L) row axis):
  - hidden dims on partitions, rows on the free dim (256 columns)
  - all 5 offsets are batched into one matmul chain via block-diagonal
    weights. The o axis splits 3+2 (h2 = 5*32 = 160 > 128 partitions);
    the B-half (offsets 3..4) lives at base partition 64 (PE quadrant
    rule: lhsT/rhs base in {0,32,64} and equal).
  - h3 rows 48:64 are memset to 1.0 and pair with nmask in mask[48] and
    B4 in W4pad[48] to fold the +B4*nmask bias term into the W4 matmul.
  - the per-row x contraction uses selection-matrix matmuls:
      xe[(c,d), r] = x[c, r]     (partition broadcast via matmul)
      prod = Msum * xe           (elementwise)
      x_new[d, r] = sum_c prod[(c,d), r]   (selection matmul)
  - matmuls run in fp32r (TF32-like, 11-bit mantissa, 4x faster than
    fp32 on the PE): weights are pre-rounded on the host, activations
    are rounded by their producing instruction writing an fp32r tile.
    End-to-end output error vs the fp32 reference is ~3e-4 of scale.
  - the two layers' MLP pipelines are independent (both depend only on
    dt); their instructions are interleaved so the PE stays dense.
"""

import numpy as np

F = 2
KW = 5  # band width (kernel size)
CIN = 8
COUT = 8
H1, H2, H3 = 16, 32, 16
NT = 100  # n_types
B, L = 4, 512
NCORES = 8
R = (B * L) // NCORES  # 256 rows per core

# offsets 0..2 are the A-half (base partition 0), 3..4 the B-half (base 64)
OA, OB = 3, 2

TRACE = False
LAST_RESULTS = None

_cache = {}


def _round_f32r(x):
    """Round-to-nearest keeping 11 mantissa bits (hardware fp32r format)."""
    b = np.ascontiguousarray(x, np.float32).view(np.uint32)
    b = (b + np.uint32(0x800)) & np.uint32(0xFFFFF000)
    return b.view(np.float32)


def _wpack_layout():
    """Weight pack (fp32r): all matmul stationary operands, both layers.

    W2B sits at base partition 64 (its rhs h1[64:96] is at base 64 and the
    PE requires equal lhsT/rhs base partitions); every other block at 0.
    Returns (cols dict name -> (P, col, W, base), total_cols).
    """
    items = [
        ("embX", NT, CIN * COUT, 0),
        ("SelX", CIN * COUT, CIN * COUT, 0),
        ("sel8", CIN * COUT, COUT, 0),
    ]
    for f in range(F):
        items += [
            (f"W1pad{f}", KW, 96, 0),          # cols 0:48 = W1A blkdiag, 64:96 = W1B
            (f"W2A{f}", OA * H1, OA * H2, 0),      # (48, 96)
            (f"W2B{f}", OB * H1, OB * H2, 64),     # (32, 64) @ base 64
            (f"W3A{f}", OA * H2, OA * H3, 0),      # (96, 48)
            (f"W3B{f}", OB * H2, OB * H3, 0),      # (64, 32)
            (f"W4pad{f}", 96, CIN * COUT, 0),      # 0:48 W4A, 48 B4, 64:96 W4B
        ]
    cols = {}
    col = 0
    for name, p, w, base in items:
        cols[name] = (p, col, w, base)
        col += w
    return cols, col


def _bpack_layout():
    """Bias pack (fp32): per-partition bias columns for the ACT/DVE stages."""
    items = []
    for f in range(F):
        items += [
            (f"B1pad{f}", 96, 1, 0),
            (f"B2A{f}", OA * H2, 1, 0),
            (f"B2B{f}", OB * H2, 1, 0),
            (f"B3A{f}", OA * H3, 1, 0),
            (f"B3B{f}", OB * H3, 1, 0),
        ]
    cols = {}
    col = 0
    for name, p, w, base in items:
        cols[name] = (p, col, w, base)
        col += w
    return cols, col


def _build_pack_arrays(emb, W1, B1, W2, B2, W3, B3, W4, B4):
    wcols, wW = _wpack_layout()
    bcols, bW = _bpack_layout()
    wpack = np.zeros((128, wW), np.float32)
    bpack = np.zeros((128, bW), np.float32)

    def put(pack, cols, name, arr):
        p, col, w, base = cols[name]
        assert arr.shape == (p, w), (name, arr.shape, (p, w))
        pack[base : base + p, col : col + w] = arr

    put(wpack, wcols, "embX", np.repeat(emb.astype(np.float32), COUT, axis=1))
    selx = np.zeros((CIN * COUT, CIN * COUT), np.float32)
    for cp in range(CIN):
        for dp in range(COUT):
            for d in range(COUT):
                selx[cp * COUT + dp, dp * COUT + d] = 1.0
    put(wpack, wcols, "SelX", selx)
    put(wpack, wcols, "sel8", np.tile(np.eye(COUT, dtype=np.float32), (CIN, 1)))

    for f in range(F):
        w1f = W1[f].reshape(H1).astype(np.float32)
        w2f = W2[f].astype(np.float32)
        w3f = W3[f].astype(np.float32)
        w4f = W4[f].astype(np.float32)

        w1p = np.zeros((KW, 96), np.float32)
        for o in range(OA):
            w1p[o, o * H1 : (o + 1) * H1] = w1f
        for o in range(OB):
            w1p[OA + o, 64 + o * H1 : 64 + (o + 1) * H1] = w1f
        put(wpack, wcols, f"W1pad{f}", w1p)
        b1p = np.zeros((96, 1), np.float32)
        b1p[0:48, 0] = np.tile(B1[f], OA)
        b1p[64:96, 0] = np.tile(B1[f], OB)
        put(bpack, bcols, f"B1pad{f}", b1p)

        w2a = np.zeros((OA * H1, OA * H2), np.float32)
        for o in range(OA):
            w2a[o * H1 : (o + 1) * H1, o * H2 : (o + 1) * H2] = w2f
        put(wpack, wcols, f"W2A{f}", w2a)
        put(bpack, bcols, f"B2A{f}", np.tile(B2[f], OA)[:, None].astype(np.float32))
        w2b = np.zeros((OB * H1, OB * H2), np.float32)
        for o in range(OB):
            w2b[o * H1 : (o + 1) * H1, o * H2 : (o + 1) * H2] = w2f
        put(wpack, wcols, f"W2B{f}", w2b)
        put(bpack, bcols, f"B2B{f}", np.tile(B2[f], OB)[:, None].astype(np.float32))

        w3a = np.zeros((OA * H2, OA * H3), np.float32)
        for o in range(OA):
            w3a[o * H2 : (o + 1) * H2, o * H3 : (o + 1) * H3] = w3f
        put(wpack, wcols, f"W3A{f}", w3a)
        put(bpack, bcols, f"B3A{f}", np.tile(B3[f], OA)[:, None].astype(np.float32))
        w3b = np.zeros((OB * H2, OB * H3), np.float32)
        for o in range(OB):
            w3b[o * H2 : (o + 1) * H2, o * H3 : (o + 1) * H3] = w3f
        put(wpack, wcols, f"W3B{f}", w3b)
        put(bpack, bcols, f"B3B{f}", np.tile(B3[f], OB)[:, None].astype(np.float32))

        w4p = np.zeros((96, CIN * COUT), np.float32)
        w4p[0:48] = np.tile(w4f, (OA, 1))
        w4p[48] = B4[f]
        w4p[64:96] = np.tile(w4f, (OB, 1))
        put(wpack, wcols, f"W4pad{f}", w4p)

    return _round_f32r(wpack), bpack


def _build_nc():
    import concourse.bacc as bacc
    import concourse.mybir as mybir
    from concourse.tile import TileContext

    F32 = mybir.dt.float32
    F32R = mybir.dt.float32r
    RELU = mybir.ActivationFunctionType.Relu
    ADD = mybir.AluOpType.add
    MAX = mybir.AluOpType.max

    wcols, wW = _wpack_layout()
    bcols, bW = _bpack_layout()

    nc = bacc.Bacc("TRN2", debug=False)
    # tvec frame (5, 512): cols 0:256 = t_i, cols 256:512 = t_{i-1-o}
    tvec_d = nc.dram_tensor("tvec", (KW, 2 * R), F32, kind="ExternalInput")
    wpack_d = nc.dram_tensor("wpack", (128, wW), F32R, kind="ExternalInput")
    bpack_d = nc.dram_tensor("bpack", (128, bW), F32, kind="ExternalInput")
    onehot_d = nc.dram_tensor("onehot", (NT, R), F32R, kind="ExternalInput")
    # mask96 rows: 0:48 = offsets 0..2 (x16), 48 = nmask, 49:64 = 0,
    # 64:96 = offsets 3..4 (x16)
    mask_d = nc.dram_tensor("mask96", (96, R), F32, kind="ExternalInput")
    out_d = nc.dram_tensor("out", (CIN, R), F32, kind="ExternalOutput")

    with TileContext(nc) as tc:
        with (
            tc.tile_pool(name="const", bufs=1) as cpool,
            tc.tile_pool(name="work", bufs=2) as wpool,
            tc.tile_pool(name="psum", bufs=2, space="PSUM") as ppool,
        ):
            # DMA order matters: tvec gates the whole MLP chain, wpack the
            # first matmuls; onehot/mask/biases are needed later.
            tvt = cpool.tile([KW, 2 * R], F32, tag="tvec")
            nc.sync.dma_start(out=tvt, in_=tvec_d.ap())
            wpack = cpool.tile([128, wW], F32R, tag="wpack")
            nc.sync.dma_start(out=wpack, in_=wpack_d.ap())
            onehot = cpool.tile([NT, R], F32R, tag="onehot")
            nc.sync.dma_start(out=onehot, in_=onehot_d.ap())
            mask96 = cpool.tile([96, R], F32, tag="mask96")
            nc.gpsimd.dma_start(out=mask96, in_=mask_d.ap())
            bpack = cpool.tile([128, bW], F32, tag="bpack")
            nc.scalar.dma_start(out=bpack, in_=bpack_d.ap())

            def wslice(name):
                p, col, w, base = wcols[name]
                return wpack[base : base + p, col : col + w]

            def bslice(name):
                p, col, w, base = bcols[name]
                return bpack[base : base + p, col : col + w]

            # dt[o, r] = t_i - t_{i-1-o} (garbage where masked; masked later)
            dt = wpool.tile([KW, R], F32R, tag="dt")
            nc.vector.tensor_sub(out=dt, in0=tvt[:, 0:R], in1=tvt[:, R : 2 * R])

            # xe[(c,d), r] = x0[c, r] = (emb gather), partition-broadcast over d
            xe_ps = ppool.tile([CIN * COUT, R], F32, tag="xe_ps", bufs=1)
            nc.tensor.matmul(xe_ps, wslice("embX"), onehot, start=True, stop=True)
            xe = wpool.tile([CIN * COUT, R], F32, tag="xe")
            nc.scalar.copy(out=xe, in_=xe_ps)

            # ---- the 5-offset MLPs of both layers, interleaved stage by
            # stage so the PE runs dense (they only depend on dt) ----
            h1ps, h1, h2psA, h2psB, h2A, h2B = {}, {}, {}, {}, {}, {}
            h3ps, h3, h3m, msum = {}, {}, {}, {}

            for f in range(F):
                h1ps[f] = ppool.tile([96, R], F32, tag="mm", bufs=5, name=f"h1ps{f}")
                nc.tensor.matmul(h1ps[f], wslice(f"W1pad{f}"), dt, start=True, stop=True)
            for f in range(F):
                h1[f] = wpool.tile([96, R], F32R, tag="h1", name=f"h1_{f}")
                nc.scalar.activation(out=h1[f], in_=h1ps[f], func=RELU, bias=bslice(f"B1pad{f}"))
            for f in range(F):
                h2psA[f] = ppool.tile([OA * H2, R], F32, tag="mm", bufs=5, name=f"h2psA{f}")
                nc.tensor.matmul(h2psA[f], wslice(f"W2A{f}"), h1[f][0 : OA * H1, :], start=True, stop=True)
                h2psB[f] = ppool.tile([OB * H2, R], F32, tag="mm", bufs=5, name=f"h2psB{f}")
                nc.tensor.matmul(h2psB[f], wslice(f"W2B{f}"), h1[f][64 : 64 + OB * H1, :], start=True, stop=True)
            for f in range(F):
                h2A[f] = wpool.tile([OA * H2, R], F32R, tag="h2A", name=f"h2A_{f}")
                nc.scalar.activation(out=h2A[f], in_=h2psA[f], func=RELU, bias=bslice(f"B2A{f}"))
                h2B[f] = wpool.tile([OB * H2, R], F32R, tag="h2B", name=f"h2B_{f}")
                nc.vector.tensor_scalar(h2B[f], h2psB[f], bslice(f"B2B{f}"), 0.0, ADD, MAX)
            for f in range(F):
                h3ps[f] = ppool.tile([96, R], F32, tag="mm", bufs=5, name=f"h3ps{f}")
                nc.tensor.matmul(h3ps[f][0 : OA * H3, :], wslice(f"W3A{f}"), h2A[f], start=True, stop=True)
                nc.tensor.matmul(h3ps[f][64 : 64 + OB * H3, :], wslice(f"W3B{f}"), h2B[f], start=True, stop=True)
            for f in range(F):
                # rows 48:64 become 1.0 (row 48 pairs with nmask/B4); memset
                # [32:64] runs before act3A overwrites [0:48]
                h3[f] = wpool.tile([96, R], F32, tag="h3", name=f"h3_{f}")
                nc.gpsimd.memset(h3[f][32:64, :], 1.0)
                nc.scalar.activation(out=h3[f][0 : OA * H3, :], in_=h3ps[f][0 : OA * H3, :], func=RELU, bias=bslice(f"B3A{f}"))
                nc.vector.tensor_scalar(h3[f][64 : 64 + OB * H3, :], h3ps[f][64 : 64 + OB * H3, :], bslice(f"B3B{f}"), 0.0, ADD, MAX)
            for f in range(F):
                # band mask (+ nmask row) folded in before the W4 matmul
                h3m[f] = wpool.tile([96, R], F32R, tag="h3m", name=f"h3m_{f}")
                nc.vector.tensor_mul(out=h3m[f], in0=h3[f], in1=mask96)
            for f in range(F):
                # Msum[(c,d), r] = sum_o kv_o + B4 * nmask, in one matmul
                msum[f] = ppool.tile([CIN * COUT, R], F32, tag="msum", bufs=2, name=f"msum{f}")
                nc.tensor.matmul(msum[f], wslice(f"W4pad{f}"), h3m[f], start=True, stop=True)

            # ---- serial x-contraction tail ----
            prod0 = wpool.tile([CIN * COUT, R], F32R, tag="prod")
            nc.vector.tensor_mul(out=prod0, in0=msum[0], in1=xe)
            xe_ps2 = ppool.tile([CIN * COUT, R], F32, tag="xe_ps", bufs=1)
            nc.tensor.matmul(xe_ps2, wslice("SelX"), prod0, start=True, stop=True)
            xe2 = wpool.tile([CIN * COUT, R], F32, tag="xe")
            nc.scalar.copy(out=xe2, in_=xe_ps2)

            prod1 = wpool.tile([CIN * COUT, R], F32R, tag="prod")
            nc.vector.tensor_mul(out=prod1, in0=msum[1], in1=xe2)
            out_ps = ppool.tile([CIN, R], F32, tag="xe_ps", bufs=1)
            nc.tensor.matmul(out_ps, wslice("sel8"), prod1, start=True, stop=True)
            xout = wpool.tile([CIN, R], F32, tag="xout")
            nc.vector.tensor_copy(out=xout, in_=out_ps)
            nc.sync.dma_start(out=out_d.ap(), in_=xout)

    nc.finalize()
    return nc


def _per_core_inputs(times, features, core):
    rows = np.arange(core * R, (core + 1) * R)
    b = rows // L
    i = rows % L

    tcur = times[b, i].astype(np.float32)
    tc5 = np.tile(tcur, (KW, 1))
    tp5 = np.zeros((KW, R), np.float32)
    mask = np.zeros((KW, R), np.float32)
    for o in range(1, KW + 1):
        valid = i >= o
        tp5[o - 1, valid] = times[b[valid], i[valid] - o]
        mask[o - 1, valid] = 1.0
    mask96 = np.zeros((96, R), np.float32)
    mask96[0 : OA * H3] = np.repeat(mask[:OA], H3, axis=0)  # partition (o*16+h)
    mask96[48] = mask.sum(axis=0)  # nmask row (pairs with ones/B4 at 48)
    mask96[64 : 64 + OB * H3] = np.repeat(mask[OA:], H3, axis=0)
    tvec = np.ascontiguousarray(np.concatenate([tc5, tp5], axis=1))  # (5, 512)

    feat = features[b, i].astype(np.int64)
    onehot = (feat[None, :] == np.arange(NT)[:, None]).astype(np.float32)
    return tvec, mask96, onehot


def kernel(times, features, emb, W1, B1, W2, B2, W3, B3, W4, B4):
    global LAST_RESULTS
    from concourse.bass_utils import run_bass_kernel_spmd

    times = np.asarray(times, dtype=np.float32)
    features = np.asarray(features)
    emb = np.asarray(emb, dtype=np.float32)
    W1, B1 = np.asarray(W1, np.float32), np.asarray(B1, np.float32)
    W2, B2 = np.asarray(W2, np.float32), np.asarray(B2, np.float32)
    W3, B3 = np.asarray(W3, np.float32), np.asarray(B3, np.float32)
    W4, B4 = np.asarray(W4, np.float32), np.asarray(B4, np.float32)

    if "nc" not in _cache:
        _cache["nc"] = _build_nc()
    nc = _cache["nc"]

    wpack, bpack = _build_pack_arrays(emb, W1, B1, W2, B2, W3, B3, W4, B4)

    in_maps = []
    for core in range(NCORES):
        tvec, mask96, onehot = _per_core_inputs(times, features, core)
        in_maps.append(
            {
                "tvec": tvec,
                "wpack": wpack,
                "bpack": bpack,
                "onehot": onehot,
                "mask96": mask96,
            }
        )

    res = run_bass_kernel_spmd(nc, in_maps, list(range(NCORES)), trace=TRACE)
    LAST_RESULTS = res

    out = np.zeros((B * L, CIN), np.float32)
    for core in range(NCORES):
        out[core * R : (core + 1) * R, :] = res.results[core]["out"].T
    return out.reshape(B, L, CIN)


# revision 34
# speedup vs baseline: 1.9547x; 1.0833x over previous
"""Trainium2 Bass kernel for nn_CCNN (banded continuous-kernel conv).

Math: the reference builds a full (B,L,L) pairwise tensor, runs a tiny
scalar->8x8-matrix MLP on every (i,j) pair, masks to the band
j in [i-5, i-1], and contracts:  x_new[b,i,:] = x[b,i,:] @ sum_j kv[b,i,j].
Only the 5 sub-diagonals survive the band mask, so we evaluate the MLP
only on the 5 offsets o=1..5 per row:  dt_o = t_i - t_{i-o}.

Layout on device (per core, R=256 rows of the flattened (B*L) row axis):
  - hidden dims on partitions, rows on the free dim (256 columns)
  - all 5 offsets are batched into one matmul chain via block-diagonal
    weights. The o axis splits 3+2 (h2 = 5*32 = 160 > 128 partitions);
    the B-half (offsets 3..4) lives at base partition 64 (PE quadrant
    rule: lhsT/rhs base in {0,32,64} and equal).
  - h3 rows 48:64 are memset to 1.0 and pair with nmask in mask[48] and
    B4 in W4pad[48] to fold the +B4*nmask bias term into the W4 matmul.
  - the per-row x contraction uses selection-matrix matmuls:
      xe[(c,d), r] = x[c, r]     (partition broadcast via matmul)
      prod = Msum * xe           (elementwise)
      x_new[d, r] = sum_c prod[(c,d), r]   (selection matmul)
  - matmuls run in fp32r (TF32-like, 11-bit mantissa, 4x faster than
    fp32 on the PE): weights are pre-rounded on the host, activations
    are rounded by their producing instruction writing an fp32r tile.
    End-to-end output error vs the fp32 reference is ~3e-4 of scale.
  - the two layers' MLP pipelines are independent (both depend only on
    dt); their instructions are interleaved so the PE stays dense.
"""

import numpy as np

F = 2
KW = 5  # band width (kernel size)
CIN = 8
COUT = 8
H1, H2, H3 = 16, 32, 16
NT = 100  # n_types
B, L = 4, 512
NCORES = 8
R = (B * L) // NCORES  # 256 rows per core

# offsets 0..2 are the A-half (base partition 0), 3..4 the B-half (base 64)
OA, OB = 3, 2

TRACE = False
LAST_RESULTS = None

_cache = {}


def _round_f32r(x):
    """Round-to-nearest keeping 11 mantissa bits (hardware fp32r format)."""
    b = np.ascontiguousarray(x, np.float32).view(np.uint32)
    b = (b + np.uint32(0x800)) & np.uint32(0xFFFFF000)
    return b.view(np.float32)


def _layer_weight_items(f):
    return [
        (f"W1pad{f}", KW, 96, 0),          # cols 0:48 = W1A blkdiag, 64:96 = W1B
        (f"W2A{f}", OA * H1, OA * H2, 0),      # (48, 96)
        (f"W2B{f}", OB * H1, OB * H2, 64),     # (32, 64) @ base 64
        (f"W3A{f}", OA * H2, OA * H3, 0),      # (96, 48)
        (f"W3B{f}", OB * H2, OB * H3, 0),      # (64, 32)
        (f"W4pad{f}", 96, CIN * COUT, 0),      # 0:48 W4A, 48 B4, 64:96 W4B
    ]


def _alloc_cols(items):
    cols = {}
    col = 0
    for name, p, w, base in items:
        cols[name] = (p, col, w, base)
        col += w
    return cols, col


def _wpack_layout():
    """Weight packs (fp32r): all matmul stationary operands.

    Split into the DMA-critical layer-0 pack (gates the first matmul) and
    the rest. W2B sits at base partition 64 (its rhs h1[64:96] is at base
    64 and the PE requires equal lhsT/rhs base partitions).
    """
    crit = _layer_weight_items(0)
    rest = [
        ("embX", NT, CIN * COUT, 0),
        ("SelX", CIN * COUT, CIN * COUT, 0),
        ("sel8", CIN * COUT, COUT, 0),
    ] + _layer_weight_items(1)
    return _alloc_cols(crit), _alloc_cols(rest)


def _bpack_layout():
    """Bias pack (fp32): per-partition bias columns for the ACT/DVE stages."""
    items = []
    for f in range(F):
        items += [
            (f"B1pad{f}", 96, 1, 0),
            (f"B2A{f}", OA * H2, 1, 0),
            (f"B2B{f}", OB * H2, 1, 0),
            (f"B3A{f}", OA * H3, 1, 0),
            (f"B3B{f}", OB * H3, 1, 0),
        ]
    cols = {}
    col = 0
    for name, p, w, base in items:
        cols[name] = (p, col, w, base)
        col += w
    return cols, col


def _build_pack_arrays(emb, W1, B1, W2, B2, W3, B3, W4, B4):
    (ccols, cW), (rcols, rW) = _wpack_layout()
    bcols, bW = _bpack_layout()
    wcrit = np.zeros((128, cW), np.float32)
    wrest = np.zeros((128, rW), np.float32)
    bpack = np.zeros((128, bW), np.float32)

    def put(pack, cols, name, arr):
        p, col, w, base = cols[name]
        assert arr.shape == (p, w), (name, arr.shape, (p, w))
        pack[base : base + p, col : col + w] = arr

    put(wrest, rcols, "embX", np.repeat(emb.astype(np.float32), COUT, axis=1))
    selx = np.zeros((CIN * COUT, CIN * COUT), np.float32)
    for cp in range(CIN):
        for dp in range(COUT):
            for d in range(COUT):
                selx[cp * COUT + dp, dp * COUT + d] = 1.0
    put(wrest, rcols, "SelX", selx)
    put(wrest, rcols, "sel8", np.tile(np.eye(COUT, dtype=np.float32), (CIN, 1)))

    for f in range(F):
        wpack, wcols = (wcrit, ccols) if f == 0 else (wrest, rcols)
        w1f = W1[f].reshape(H1).astype(np.float32)
        w2f = W2[f].astype(np.float32)
        w3f = W3[f].astype(np.float32)
        w4f = W4[f].astype(np.float32)

        w1p = np.zeros((KW, 96), np.float32)
        for o in range(OA):
            w1p[o, o * H1 : (o + 1) * H1] = w1f
        for o in range(OB):
            w1p[OA + o, 64 + o * H1 : 64 + (o + 1) * H1] = w1f
        put(wpack, wcols, f"W1pad{f}", w1p)
        b1p = np.zeros((96, 1), np.float32)
        b1p[0:48, 0] = np.tile(B1[f], OA)
        b1p[64:96, 0] = np.tile(B1[f], OB)
        put(bpack, bcols, f"B1pad{f}", b1p)

        w2a = np.zeros((OA * H1, OA * H2), np.float32)
        for o in range(OA):
            w2a[o * H1 : (o + 1) * H1, o * H2 : (o + 1) * H2] = w2f
        put(wpack, wcols, f"W2A{f}", w2a)
        put(bpack, bcols, f"B2A{f}", np.tile(B2[f], OA)[:, None].astype(np.float32))
        w2b = np.zeros((OB * H1, OB * H2), np.float32)
        for o in range(OB):
            w2b[o * H1 : (o + 1) * H1, o * H2 : (o + 1) * H2] = w2f
        put(wpack, wcols, f"W2B{f}", w2b)
        put(bpack, bcols, f"B2B{f}", np.tile(B2[f], OB)[:, None].astype(np.float32))

        w3a = np.zeros((OA * H2, OA * H3), np.float32)
        for o in range(OA):
            w3a[o * H2 : (o + 1) * H2, o * H3 : (o + 1) * H3] = w3f
        put(wpack, wcols, f"W3A{f}", w3a)
        put(bpack, bcols, f"B3A{f}", np.tile(B3[f], OA)[:, None].astype(np.float32))
        w3b = np.zeros((OB * H2, OB * H3), np.float32)
        for o in range(OB):
            w3b[o * H2 : (o + 1) * H2, o * H3 : (o + 1) * H3] = w3f
        put(wpack, wcols, f"W3B{f}", w3b)
        put(bpack, bcols, f"B3B{f}", np.tile(B3[f], OB)[:, None].astype(np.float32))

        w4p = np.zeros((96, CIN * COUT), np.float32)
        w4p[0:48] = np.tile(w4f, (OA, 1))
        w4p[48] = B4[f]
        w4p[64:96] = np.tile(w4f, (OB, 1))
        put(wpack, wcols, f"W4pad{f}", w4p)

    return _round_f32r(wcrit), _round_f32r(wrest), bpack


def _build_nc():
    import concourse.bacc as bacc
    import concourse.mybir as mybir
    from concourse.tile import TileContext

    F32 = mybir.dt.float32
    F32R = mybir.dt.float32r
    RELU = mybir.ActivationFunctionType.Relu
    ADD = mybir.AluOpType.add
    MAX = mybir.AluOpType.max

    (ccols, cW), (rcols, rW) = _wpack_layout()
    bcols, bW = _bpack_layout()

    nc = bacc.Bacc("TRN2", debug=False)
    # tvec frame (5, 512): cols 0:256 = t_i, cols 256:512 = t_{i-1-o}
    tvec_d = nc.dram_tensor("tvec", (KW, 2 * R), F32, kind="ExternalInput")
    wcrit_d = nc.dram_tensor("wcrit", (128, cW), F32R, kind="ExternalInput")
    wrest_d = nc.dram_tensor("wrest", (128, rW), F32R, kind="ExternalInput")
    bpack_d = nc.dram_tensor("bpack", (128, bW), F32, kind="ExternalInput")
    onehot_d = nc.dram_tensor("onehot", (NT, R), F32R, kind="ExternalInput")
    # mask96 rows: 0:48 = offsets 0..2 (x16), 48 = nmask, 49:64 = 0,
    # 64:96 = offsets 3..4 (x16)
    mask_d = nc.dram_tensor("mask96", (96, R), F32, kind="ExternalInput")
    out_d = nc.dram_tensor("out", (CIN, R), F32, kind="ExternalOutput")

    with TileContext(nc) as tc:
        with (
            tc.tile_pool(name="const", bufs=1) as cpool,
            tc.tile_pool(name="work", bufs=2) as wpool,
            tc.tile_pool(name="psum", bufs=2, space="PSUM") as ppool,
        ):
            # DMA order matters: the HWDGE transfers serialize in dispatch
            # order, so the chain-gating tensors (tvec, layer-0 weights) go
            # first on SP; mask/bias ride the SWDGE (Pool) queue; onehot
            # (needed latest) goes on the ACT queue.
            tvt = cpool.tile([KW, 2 * R], F32, tag="tvec")
            nc.sync.dma_start(out=tvt, in_=tvec_d.ap())
            wcrit = cpool.tile([128, cW], F32R, tag="wcrit")
            nc.sync.dma_start(out=wcrit, in_=wcrit_d.ap())
            wrest = cpool.tile([128, rW], F32R, tag="wrest")
            nc.sync.dma_start(out=wrest, in_=wrest_d.ap())
            mask96 = cpool.tile([96, R], F32, tag="mask96")
            nc.gpsimd.dma_start(out=mask96, in_=mask_d.ap())
            bpack = cpool.tile([128, bW], F32, tag="bpack")
            nc.gpsimd.dma_start(out=bpack, in_=bpack_d.ap())
            onehot = cpool.tile([NT, R], F32R, tag="onehot")
            nc.scalar.dma_start(out=onehot, in_=onehot_d.ap())

            wtiles = {0: (wcrit, ccols), 1: (wrest, rcols)}

            def wslice(name):
                f = 0 if name in ccols else 1
                pk, cols = wtiles[f]
                p, col, w, base = cols[name]
                return pk[base : base + p, col : col + w]

            def bslice(name):
                p, col, w, base = bcols[name]
                return bpack[base : base + p, col : col + w]

            # dt[o, r] = t_i - t_{i-1-o} (garbage where masked; masked later)
            dt = wpool.tile([KW, R], F32R, tag="dt")
            nc.vector.tensor_sub(out=dt, in0=tvt[:, 0:R], in1=tvt[:, R : 2 * R])

            # xe[(c,d), r] = x0[c, r] = (emb gather), partition-broadcast over d
            xe_ps = ppool.tile([CIN * COUT, R], F32, tag="xe_ps", bufs=1)
            nc.tensor.matmul(xe_ps, wslice("embX"), onehot, start=True, stop=True)
            xe = wpool.tile([CIN * COUT, R], F32, tag="xe")
            nc.scalar.copy(out=xe, in_=xe_ps)

            # ---- the 5-offset MLPs of both layers, interleaved stage by
            # stage so the PE runs dense (they only depend on dt) ----
            h1ps, h1, h2psA, h2psB, h2A, h2B = {}, {}, {}, {}, {}, {}
            h3ps, h3, h3m, msum = {}, {}, {}, {}

            for f in range(F):
                h1ps[f] = ppool.tile([96, R], F32, tag="mm", bufs=5, name=f"h1ps{f}")
                nc.tensor.matmul(h1ps[f], wslice(f"W1pad{f}"), dt, start=True, stop=True)
            for f in range(F):
                h1[f] = wpool.tile([96, R], F32R, tag="h1", name=f"h1_{f}")
                nc.scalar.activation(out=h1[f], in_=h1ps[f], func=RELU, bias=bslice(f"B1pad{f}"))
            for f in range(F):
                h2psA[f] = ppool.tile([OA * H2, R], F32, tag="mm", bufs=5, name=f"h2psA{f}")
                nc.tensor.matmul(h2psA[f], wslice(f"W2A{f}"), h1[f][0 : OA * H1, :], start=True, stop=True)
                h2psB[f] = ppool.tile([OB * H2, R], F32, tag="mm", bufs=5, name=f"h2psB{f}")
                nc.tensor.matmul(h2psB[f], wslice(f"W2B{f}"), h1[f][64 : 64 + OB * H1, :], start=True, stop=True)
            for f in range(F):
                h2A[f] = wpool.tile([OA * H2, R], F32R, tag="h2A", name=f"h2A_{f}")
                nc.scalar.activation(out=h2A[f], in_=h2psA[f], func=RELU, bias=bslice(f"B2A{f}"))
                h2B[f] = wpool.tile([OB * H2, R], F32R, tag="h2B", name=f"h2B_{f}")
                nc.vector.tensor_scalar(h2B[f], h2psB[f], bslice(f"B2B{f}"), 0.0, ADD, MAX)
            for f in range(F):
                h3ps[f] = ppool.tile([96, R], F32, tag="mm", bufs=5, name=f"h3ps{f}")
                nc.tensor.matmul(h3ps[f][0 : OA * H3, :], wslice(f"W3A{f}"), h2A[f], start=True, stop=True)
                nc.tensor.matmul(h3ps[f][64 : 64 + OB * H3, :], wslice(f"W3B{f}"), h2B[f], start=True, stop=True)
            for f in range(F):
                # rows 48:64 become 1.0 (row 48 pairs with nmask/B4); memset
                # [32:64] runs before act3A overwrites [0:48]
                h3[f] = wpool.tile([96, R], F32, tag="h3", name=f"h3_{f}")
                nc.gpsimd.memset(h3[f][32:64, :], 1.0)
                nc.scalar.activation(out=h3[f][0 : OA * H3, :], in_=h3ps[f][0 : OA * H3, :], func=RELU, bias=bslice(f"B3A{f}"))
                nc.vector.tensor_scalar(h3[f][64 : 64 + OB * H3, :], h3ps[f][64 : 64 + OB * H3, :], bslice(f"B3B{f}"), 0.0, ADD, MAX)
            for f in range(F):
                # band mask (+ nmask row) folded in before the W4 matmul
                h3m[f] = wpool.tile([96, R], F32R, tag="h3m", name=f"h3m_{f}")
                nc.vector.tensor_mul(out=h3m[f], in0=h3[f], in1=mask96)
            for f in range(F):
                # Msum[(c,d), r] = sum_o kv_o + B4 * nmask, in one matmul
                msum[f] = ppool.tile([CIN * COUT, R], F32, tag="msum", bufs=2, name=f"msum{f}")
                nc.tensor.matmul(msum[f], wslice(f"W4pad{f}"), h3m[f], start=True, stop=True)

            # ---- serial x-contraction tail ----
            prod0 = wpool.tile([CIN * COUT, R], F32R, tag="prod")
            nc.vector.tensor_mul(out=prod0, in0=msum[0], in1=xe)
            xe_ps2 = ppool.tile([CIN * COUT, R], F32, tag="xe_ps", bufs=1)
            nc.tensor.matmul(xe_ps2, wslice("SelX"), prod0, start=True, stop=True)
            xe2 = wpool.tile([CIN * COUT, R], F32, tag="xe")
            # DVE copy: keeps the copy->prod1 handoff on one engine
            nc.vector.tensor_copy(out=xe2, in_=xe_ps2)

            prod1 = wpool.tile([CIN * COUT, R], F32R, tag="prod")
            nc.vector.tensor_mul(out=prod1, in0=msum[1], in1=xe2)
            out_ps = ppool.tile([CIN, R], F32, tag="xe_ps", bufs=1)
            nc.tensor.matmul(out_ps, wslice("sel8"), prod1, start=True, stop=True)
            xout = wpool.tile([CIN, R], F32, tag="xout")
            nc.vector.tensor_copy(out=xout, in_=out_ps)
            nc.sync.dma_start(out=out_d.ap(), in_=xout)

    nc.finalize()
    return nc


def _per_core_inputs(times, features, core):
    rows = np.arange(core * R, (core + 1) * R)
    b = rows // L
    i = rows % L

    tcur = times[b, i].astype(np.float32)
    tc5 = np.tile(tcur, (KW, 1))
    tp5 = np.zeros((KW, R), np.float32)
    mask = np.zeros((KW, R), np.float32)
    for o in range(1, KW + 1):
        valid = i >= o
        tp5[o - 1, valid] = times[b[valid], i[valid] - o]
        mask[o - 1, valid] = 1.0
    mask96 = np.zeros((96, R), np.float32)
    mask96[0 : OA * H3] = np.repeat(mask[:OA], H3, axis=0)  # partition (o*16+h)
    mask96[48] = mask.sum(axis=0)  # nmask row (pairs with ones/B4 at 48)
    mask96[64 : 64 + OB * H3] = np.repeat(mask[OA:], H3, axis=0)
    tvec = np.ascontiguousarray(np.concatenate([tc5, tp5], axis=1))  # (5, 512)

    feat = features[b, i].astype(np.int64)
    onehot = (feat[None, :] == np.arange(NT)[:, None]).astype(np.float32)
    return tvec, mask96, onehot


def kernel(times, features, emb, W1, B1, W2, B2, W3, B3, W4, B4):
    global LAST_RESULTS
    from concourse.bass_utils import run_bass_kernel_spmd

    times = np.asarray(times, dtype=np.float32)
    features = np.asarray(features)
    emb = np.asarray(emb, dtype=np.float32)
    W1, B1 = np.asarray(W1, np.float32), np.asarray(B1, np.float32)
    W2, B2 = np.asarray(W2, np.float32), np.asarray(B2, np.float32)
    W3, B3 = np.asarray(W3, np.float32), np.asarray(B3, np.float32)
    W4, B4 = np.asarray(W4, np.float32), np.asarray(B4, np.float32)

    if "nc" not in _cache:
        _cache["nc"] = _build_nc()
    nc = _cache["nc"]

    wcrit, wrest, bpack = _build_pack_arrays(emb, W1, B1, W2, B2, W3, B3, W4, B4)

    in_maps = []
    for core in range(NCORES):
        tvec, mask96, onehot = _per_core_inputs(times, features, core)
        in_maps.append(
            {
                "tvec": tvec,
                "wcrit": wcrit,
                "wrest": wrest,
                "bpack": bpack,
                "onehot": onehot,
                "mask96": mask96,
            }
        )

    res = run_bass_kernel_spmd(nc, in_maps, list(range(NCORES)), trace=TRACE)
    LAST_RESULTS = res

    out = np.zeros((B * L, CIN), np.float32)
    for core in range(NCORES):
        out[core * R : (core + 1) * R, :] = res.results[core]["out"].T
    return out.reshape(B, L, CIN)


# revision 41
# speedup vs baseline: 2.0117x; 1.0291x over previous
"""Trainium2 Bass kernel for nn_CCNN (banded continuous-kernel conv).

Math: the reference builds a full (B,L,L) pairwise tensor, runs a tiny
scalar->8x8-matrix MLP on every (i,j) pair, masks to the band
j in [i-5, i-1], and contracts:  x_new[b,i,:] = x[b,i,:] @ sum_j kv[b,i,j].
Only the 5 sub-diagonals survive the band mask, so we evaluate the MLP
only on the 5 offsets o=1..5 per row:  dt_o = t_i - t_{i-o}.

Layout on device (per core, R=256 rows of the flattened (B*L) row axis):
  - hidden dims on partitions, rows on the free dim (256 columns)
  - all 5 offsets are batched into one matmul chain via block-diagonal
    weights. The o axis splits 3+2 (h2 = 5*32 = 160 > 128 partitions);
    the B-half (offsets 3..4) lives at base partition 64 (PE quadrant
    rule: lhsT/rhs base in {0,32,64} and equal).
  - h3 rows 48:64 are memset to 1.0 and pair with nmask in mask[48] and
    B4 in W4pad[48] to fold the +B4*nmask bias term into the W4 matmul.
  - the per-row x contraction uses selection-matrix matmuls:
      xe[(c,d), r] = x[c, r]     (partition broadcast via matmul)
      prod = Msum * xe           (elementwise)
      x_new[d, r] = sum_c prod[(c,d), r]   (selection matmul)
  - matmuls run in fp32r (TF32-like, 11-bit mantissa, 4x faster than
    fp32 on the PE): weights are pre-rounded on the host, activations
    are rounded by their producing instruction writing an fp32r tile.
    End-to-end output error vs the fp32 reference is ~3e-4 of scale.
  - the two layers' MLP pipelines are independent (both depend only on
    dt); their instructions are interleaved so the PE stays dense.
"""

import numpy as np

F = 2
KW = 5  # band width (kernel size)
CIN = 8
COUT = 8
H1, H2, H3 = 16, 32, 16
NT = 100  # n_types
B, L = 4, 512
NCORES = 8
R = (B * L) // NCORES  # 256 rows per core

# offsets 0..2 are the A-half (base partition 0), 3..4 the B-half (base 64)
OA, OB = 3, 2

TRACE = False
LAST_RESULTS = None

_cache = {}


def _round_f32r(x):
    """Round-to-nearest keeping 11 mantissa bits (hardware fp32r format)."""
    b = np.ascontiguousarray(x, np.float32).view(np.uint32)
    b = (b + np.uint32(0x800)) & np.uint32(0xFFFFF000)
    return b.view(np.float32)


def _layer_weight_items(f):
    return [
        (f"W1pad{f}", KW, 96, 0),          # cols 0:48 = W1A blkdiag, 64:96 = W1B
        (f"W2A{f}", OA * H1, OA * H2, 0),      # (48, 96)
        (f"W2B{f}", OB * H1, OB * H2, 64),     # (32, 64) @ base 64
        (f"W3A{f}", OA * H2, OA * H3, 0),      # (96, 48)
        (f"W3B{f}", OB * H2, OB * H3, 0),      # (64, 32)
        (f"W4pad{f}", 96, CIN * COUT, 0),      # 0:48 W4A, 48 B4, 64:96 W4B
    ]


def _alloc_cols(items):
    cols = {}
    col = 0
    for name, p, w, base in items:
        cols[name] = (p, col, w, base)
        col += w
    return cols, col


def _wpack_layout():
    """Weight packs (fp32r): all matmul stationary operands.

    Split into three DMA units ordered by when the pipeline needs them:
    layer-0 weights (gate the first matmuls), layer-1 weights, selection
    matrices. W2B sits at base partition 64 (its rhs h1[64:96] is at base
    64 and the PE requires equal lhsT/rhs base partitions).
    """
    wsel = [
        ("embX", NT, CIN * COUT, 0),
        ("SelX", CIN * COUT, CIN * COUT, 0),
        ("sel8", CIN * COUT, COUT, 0),
    ]
    return (
        _alloc_cols(_layer_weight_items(0)),
        _alloc_cols(_layer_weight_items(1)),
        _alloc_cols(wsel),
    )


def _bpack_layout():
    """Bias pack (fp32): per-partition bias columns for the ACT/DVE stages."""
    items = []
    for f in range(F):
        items += [
            (f"B1pad{f}", 96, 1, 0),
            (f"B2A{f}", OA * H2, 1, 0),
            (f"B2B{f}", OB * H2, 1, 0),
            (f"B3A{f}", OA * H3, 1, 0),
            (f"B3B{f}", OB * H3, 1, 0),
        ]
    cols = {}
    col = 0
    for name, p, w, base in items:
        cols[name] = (p, col, w, base)
        col += w
    return cols, col


def _build_pack_arrays(emb, W1, B1, W2, B2, W3, B3, W4, B4):
    (c0, W0), (c1, W1c), (cs, Ws) = _wpack_layout()
    bcols, bW = _bpack_layout()
    wl0 = np.zeros((128, W0), np.float32)
    wl1 = np.zeros((128, W1c), np.float32)
    wsel = np.zeros((128, Ws), np.float32)
    bpack = np.zeros((128, bW), np.float32)

    def put(pack, cols, name, arr):
        p, col, w, base = cols[name]
        assert arr.shape == (p, w), (name, arr.shape, (p, w))
        pack[base : base + p, col : col + w] = arr

    put(wsel, cs, "embX", np.repeat(emb.astype(np.float32), COUT, axis=1))
    selx = np.zeros((CIN * COUT, CIN * COUT), np.float32)
    for cp in range(CIN):
        for dp in range(COUT):
            for d in range(COUT):
                selx[cp * COUT + dp, dp * COUT + d] = 1.0
    put(wsel, cs, "SelX", selx)
    put(wsel, cs, "sel8", np.tile(np.eye(COUT, dtype=np.float32), (CIN, 1)))

    for f in range(F):
        wpack, wcols = (wl0, c0) if f == 0 else (wl1, c1)
        w1f = W1[f].reshape(H1).astype(np.float32)
        w2f = W2[f].astype(np.float32)
        w3f = W3[f].astype(np.float32)
        w4f = W4[f].astype(np.float32)

        w1p = np.zeros((KW, 96), np.float32)
        for o in range(OA):
            w1p[o, o * H1 : (o + 1) * H1] = w1f
        for o in range(OB):
            w1p[OA + o, 64 + o * H1 : 64 + (o + 1) * H1] = w1f
        put(wpack, wcols, f"W1pad{f}", w1p)
        b1p = np.zeros((96, 1), np.float32)
        b1p[0:48, 0] = np.tile(B1[f], OA)
        b1p[64:96, 0] = np.tile(B1[f], OB)
        put(bpack, bcols, f"B1pad{f}", b1p)

        w2a = np.zeros((OA * H1, OA * H2), np.float32)
        for o in range(OA):
            w2a[o * H1 : (o + 1) * H1, o * H2 : (o + 1) * H2] = w2f
        put(wpack, wcols, f"W2A{f}", w2a)
        put(bpack, bcols, f"B2A{f}", np.tile(B2[f], OA)[:, None].astype(np.float32))
        w2b = np.zeros((OB * H1, OB * H2), np.float32)
        for o in range(OB):
            w2b[o * H1 : (o + 1) * H1, o * H2 : (o + 1) * H2] = w2f
        put(wpack, wcols, f"W2B{f}", w2b)
        put(bpack, bcols, f"B2B{f}", np.tile(B2[f], OB)[:, None].astype(np.float32))

        w3a = np.zeros((OA * H2, OA * H3), np.float32)
        for o in range(OA):
            w3a[o * H2 : (o + 1) * H2, o * H3 : (o + 1) * H3] = w3f
        put(wpack, wcols, f"W3A{f}", w3a)
        put(bpack, bcols, f"B3A{f}", np.tile(B3[f], OA)[:, None].astype(np.float32))
        w3b = np.zeros((OB * H2, OB * H3), np.float32)
        for o in range(OB):
            w3b[o * H2 : (o + 1) * H2, o * H3 : (o + 1) * H3] = w3f
        put(wpack, wcols, f"W3B{f}", w3b)
        put(bpack, bcols, f"B3B{f}", np.tile(B3[f], OB)[:, None].astype(np.float32))

        w4p = np.zeros((96, CIN * COUT), np.float32)
        w4p[0:48] = np.tile(w4f, (OA, 1))
        w4p[48] = B4[f]
        w4p[64:96] = np.tile(w4f, (OB, 1))
        put(wpack, wcols, f"W4pad{f}", w4p)

    return _round_f32r(wl0), _round_f32r(wl1), _round_f32r(wsel), bpack


def _build_nc():
    import concourse.bacc as bacc
    import concourse.mybir as mybir
    from concourse.tile import TileContext

    F32 = mybir.dt.float32
    F32R = mybir.dt.float32r
    RELU = mybir.ActivationFunctionType.Relu
    ADD = mybir.AluOpType.add
    MAX = mybir.AluOpType.max

    (c0, W0), (c1, W1c), (cs, Ws) = _wpack_layout()
    bcols, bW = _bpack_layout()

    nc = bacc.Bacc("TRN2", debug=False)
    # tvec frame (5, 512): cols 0:256 = t_i, cols 256:512 = t_{i-1-o}
    tvec_d = nc.dram_tensor("tvec", (KW, 2 * R), F32, kind="ExternalInput")
    wl0_d = nc.dram_tensor("wl0", (128, W0), F32R, kind="ExternalInput")
    wl1_d = nc.dram_tensor("wl1", (128, W1c), F32R, kind="ExternalInput")
    wsel_d = nc.dram_tensor("wsel", (128, Ws), F32R, kind="ExternalInput")
    bpack_d = nc.dram_tensor("bpack", (128, bW), F32, kind="ExternalInput")
    onehot_d = nc.dram_tensor("onehot", (NT, R), F32R, kind="ExternalInput")
    # mask96 rows: 0:48 = offsets 0..2 (x16), 48 = nmask, 49:64 = 0,
    # 64:96 = offsets 3..4 (x16)
    mask_d = nc.dram_tensor("mask96", (96, R), F32, kind="ExternalInput")
    out_d = nc.dram_tensor("out", (CIN, R), F32, kind="ExternalOutput")

    with TileContext(nc) as tc:
        with (
            tc.tile_pool(name="const", bufs=1) as cpool,
            tc.tile_pool(name="work", bufs=2) as wpool,
            tc.tile_pool(name="psum", bufs=2, space="PSUM") as ppool,
        ):
            # DMA order matters: the HWDGE transfers serialize in dispatch
            # order, so the chain-gating tensors (tvec, layer-0 weights,
            # layer-1 weights) go first on SP; mask/bias ride the SWDGE
            # (Pool) queue; onehot goes on the ACT queue.
            tvt = cpool.tile([KW, 2 * R], F32, tag="tvec")
            nc.sync.dma_start(out=tvt, in_=tvec_d.ap())
            wl0 = cpool.tile([128, W0], F32R, tag="wl0")
            nc.sync.dma_start(out=wl0, in_=wl0_d.ap())
            wl1 = cpool.tile([128, W1c], F32R, tag="wl1")
            nc.sync.dma_start(out=wl1, in_=wl1_d.ap())
            wsel = cpool.tile([128, Ws], F32R, tag="wsel")
            nc.sync.dma_start(out=wsel, in_=wsel_d.ap())
            mask96 = cpool.tile([96, R], F32, tag="mask96")
            nc.gpsimd.dma_start(out=mask96, in_=mask_d.ap())
            bpack = cpool.tile([128, bW], F32, tag="bpack")
            nc.gpsimd.dma_start(out=bpack, in_=bpack_d.ap())
            onehot = cpool.tile([NT, R], F32R, tag="onehot")
            nc.scalar.dma_start(out=onehot, in_=onehot_d.ap())

            def wslice(name):
                for pk, cols in ((wl0, c0), (wl1, c1), (wsel, cs)):
                    if name in cols:
                        p, col, w, base = cols[name]
                        return pk[base : base + p, col : col + w]
                raise KeyError(name)

            def bslice(name):
                p, col, w, base = bcols[name]
                return bpack[base : base + p, col : col + w]

            # dt[o, r] = t_i - t_{i-1-o} (garbage where masked; masked later)
            dt = wpool.tile([KW, R], F32R, tag="dt")
            nc.vector.tensor_sub(out=dt, in0=tvt[:, 0:R], in1=tvt[:, R : 2 * R])

            # ---- the 5-offset MLPs of both layers, interleaved stage by
            # stage so the PE runs dense (they only depend on dt) ----
            h1ps, h1, h2psA, h2psB, h2A, h2B = {}, {}, {}, {}, {}, {}
            h3ps, h3, h3m, msum = {}, {}, {}, {}

            for f in range(F):
                h1ps[f] = ppool.tile([96, R], F32, tag="mm", bufs=5, name=f"h1ps{f}")
                nc.tensor.matmul(h1ps[f], wslice(f"W1pad{f}"), dt, start=True, stop=True)
            for f in range(F):
                h1[f] = wpool.tile([96, R], F32R, tag="h1", name=f"h1_{f}")
                nc.scalar.activation(out=h1[f], in_=h1ps[f], func=RELU, bias=bslice(f"B1pad{f}"))
            for f in range(F):
                h2psA[f] = ppool.tile([OA * H2, R], F32, tag="mm", bufs=5, name=f"h2psA{f}")
                nc.tensor.matmul(h2psA[f], wslice(f"W2A{f}"), h1[f][0 : OA * H1, :], start=True, stop=True)
                h2psB[f] = ppool.tile([OB * H2, R], F32, tag="mm", bufs=5, name=f"h2psB{f}")
                nc.tensor.matmul(h2psB[f], wslice(f"W2B{f}"), h1[f][64 : 64 + OB * H1, :], start=True, stop=True)
            for f in range(F):
                h2A[f] = wpool.tile([OA * H2, R], F32R, tag="h2A", name=f"h2A_{f}")
                nc.scalar.activation(out=h2A[f], in_=h2psA[f], func=RELU, bias=bslice(f"B2A{f}"))
                h2B[f] = wpool.tile([OB * H2, R], F32R, tag="h2B", name=f"h2B_{f}")
                nc.vector.tensor_scalar(h2B[f], h2psB[f], bslice(f"B2B{f}"), 0.0, ADD, MAX)
            for f in range(F):
                h3ps[f] = ppool.tile([96, R], F32, tag="mm", bufs=5, name=f"h3ps{f}")
                nc.tensor.matmul(h3ps[f][0 : OA * H3, :], wslice(f"W3A{f}"), h2A[f], start=True, stop=True)
                nc.tensor.matmul(h3ps[f][64 : 64 + OB * H3, :], wslice(f"W3B{f}"), h2B[f], start=True, stop=True)
            for f in range(F):
                # rows 48:64 become 1.0 (row 48 pairs with nmask/B4); memset
                # [32:64] runs before act3A overwrites [0:48]
                h3[f] = wpool.tile([96, R], F32, tag="h3", name=f"h3_{f}")
                nc.gpsimd.memset(h3[f][32:64, :], 1.0)
                nc.scalar.activation(out=h3[f][0 : OA * H3, :], in_=h3ps[f][0 : OA * H3, :], func=RELU, bias=bslice(f"B3A{f}"))
                nc.vector.tensor_scalar(h3[f][64 : 64 + OB * H3, :], h3ps[f][64 : 64 + OB * H3, :], bslice(f"B3B{f}"), 0.0, ADD, MAX)
            # xe[(c,d), r] = x0[c, r] = (emb gather), partition-broadcast
            # over d. Emitted late: the PE runs in order, and this matmul
            # depends on the last-arriving DMAs (wsel + onehot).
            xe_ps = ppool.tile([CIN * COUT, R], F32, tag="xe_ps", bufs=1)
            nc.tensor.matmul(xe_ps, wslice("embX"), onehot, start=True, stop=True)
            xe = wpool.tile([CIN * COUT, R], F32, tag="xe")
            nc.scalar.copy(out=xe, in_=xe_ps)

            for f in range(F):
                # band mask (+ nmask row) folded in before the W4 matmul
                h3m[f] = wpool.tile([96, R], F32R, tag="h3m", name=f"h3m_{f}")
                nc.vector.tensor_mul(out=h3m[f], in0=h3[f], in1=mask96)
            for f in range(F):
                # Msum[(c,d), r] = sum_o kv_o + B4 * nmask, in one matmul
                msum[f] = ppool.tile([CIN * COUT, R], F32, tag="msum", bufs=2, name=f"msum{f}")
                nc.tensor.matmul(msum[f], wslice(f"W4pad{f}"), h3m[f], start=True, stop=True)

            # ---- serial x-contraction tail ----
            prod0 = wpool.tile([CIN * COUT, R], F32R, tag="prod")
            nc.vector.tensor_mul(out=prod0, in0=msum[0], in1=xe)
            xe_ps2 = ppool.tile([CIN * COUT, R], F32, tag="xe_ps", bufs=1)
            nc.tensor.matmul(xe_ps2, wslice("SelX"), prod0, start=True, stop=True)
            xe2 = wpool.tile([CIN * COUT, R], F32, tag="xe")
            # DVE copy: keeps the copy->prod1 handoff on one engine
            nc.vector.tensor_copy(out=xe2, in_=xe_ps2)

            prod1 = wpool.tile([CIN * COUT, R], F32R, tag="prod")
            nc.vector.tensor_mul(out=prod1, in0=msum[1], in1=xe2)
            out_ps = ppool.tile([CIN, R], F32, tag="xe_ps", bufs=1)
            nc.tensor.matmul(out_ps, wslice("sel8"), prod1, start=True, stop=True)
            xout = wpool.tile([CIN, R], F32, tag="xout")
            nc.vector.tensor_copy(out=xout, in_=out_ps)
            nc.sync.dma_start(out=out_d.ap(), in_=xout)

    nc.finalize()
    return nc


def _per_core_inputs(times, features, core):
    rows = np.arange(core * R, (core + 1) * R)
    b = rows // L
    i = rows % L

    tcur = times[b, i].astype(np.float32)
    tc5 = np.tile(tcur, (KW, 1))
    tp5 = np.zeros((KW, R), np.float32)
    mask = np.zeros((KW, R), np.float32)
    for o in range(1, KW + 1):
        valid = i >= o
        tp5[o - 1, valid] = times[b[valid], i[valid] - o]
        mask[o - 1, valid] = 1.0
    mask96 = np.zeros((96, R), np.float32)
    mask96[0 : OA * H3] = np.repeat(mask[:OA], H3, axis=0)  # partition (o*16+h)
    mask96[48] = mask.sum(axis=0)  # nmask row (pairs with ones/B4 at 48)
    mask96[64 : 64 + OB * H3] = np.repeat(mask[OA:], H3, axis=0)
    tvec = np.ascontiguousarray(np.concatenate([tc5, tp5], axis=1))  # (5, 512)

    feat = features[b, i].astype(np.int64)
    onehot = (feat[None, :] == np.arange(NT)[:, None]).astype(np.float32)
    return tvec, mask96, onehot


def kernel(times, features, emb, W1, B1, W2, B2, W3, B3, W4, B4):
    global LAST_RESULTS
    from concourse.bass_utils import run_bass_kernel_spmd

    times = np.asarray(times, dtype=np.float32)
    features = np.asarray(features)
    emb = np.asarray(emb, dtype=np.float32)
    W1, B1 = np.asarray(W1, np.float32), np.asarray(B1, np.float32)
    W2, B2 = np.asarray(W2, np.float32), np.asarray(B2, np.float32)
    W3, B3 = np.asarray(W3, np.float32), np.asarray(B3, np.float32)
    W4, B4 = np.asarray(W4, np.float32), np.asarray(B4, np.float32)

    if "nc" not in _cache:
        _cache["nc"] = _build_nc()
    nc = _cache["nc"]

    wl0, wl1, wsel, bpack = _build_pack_arrays(emb, W1, B1, W2, B2, W3, B3, W4, B4)

    in_maps = []
    for core in range(NCORES):
        tvec, mask96, onehot = _per_core_inputs(times, features, core)
        in_maps.append(
            {
                "tvec": tvec,
                "wl0": wl0,
                "wl1": wl1,
                "wsel": wsel,
                "bpack": bpack,
                "onehot": onehot,
                "mask96": mask96,
            }
        )

    res = run_bass_kernel_spmd(nc, in_maps, list(range(NCORES)), trace=TRACE)
    LAST_RESULTS = res

    out = np.zeros((B * L, CIN), np.float32)
    for core in range(NCORES):
        out[core * R : (core + 1) * R, :] = res.results[core]["out"].T
    return out.reshape(B, L, CIN)
